# revision 48
# baseline (speedup 1.0000x reference)
"""Trainium2 Bass kernel for nn_Attention (dense transformer attention over 32x32 fmap).

Math (per batch):
    qkv = w_qkv @ fmap_flat            # [1536, 1024] = [1536,512] @ [512,1024]
    q, k, v per head: [128, 1024] in (d, s) layout
    emb[s, d] = height[x] + width[y];  s = 32*x + y
    sim = (q^T (k + emb^T)) * scale    # scale folded into q weights on host
    out[h*128+d, s] = softmax_j(sim)^T V  computed as O^T = V_jd^T @ expS_T / denom

Sharding: data-parallel over batch, 2 batches per core on 8 cores. No collectives.

Schedule: the ACT engine's exp stream (64 x [128,1024] tiles, ~73us) is the
second-largest engine load after the PE (~102us), so the kernel is built as a
single software pipeline in which the QKV GEMM is chopped into [128,512]-output
chunks and interleaved into the per-head attention windows: during head h's PV
stream the PE also computes head h+1's S tiles (feeding ACT) and a budget of
QKV chunks for the next batch. ACT runs nothing but exp; PSUM evictions go to
DVE, the softmax-denominator adder tree is split across DVE and GPSIMD, and
the 1/denom broadcast is multiplied straight out of PSUM.
"""
import numpy as np
import ml_dtypes

import concourse.bass as bass
import concourse.mybir as mybir
from concourse import bacc
import concourse.tile as tile

F32 = mybir.dt.float32
F32R = mybir.dt.float32r
BF16 = mybir.dt.bfloat16
AF = mybir.ActivationFunctionType
MUL = mybir.AluOpType.mult

WARMUP_MM = 40  # junk matmuls to flip the HAM clock gate to 8/8 during input DMA

# fp8 (e4m3) DoubleRow path for the Q/K projection GEMM: packs the C=512
# contraction as 2 fp8 values per PE cell (virtual 128x256 array), ~1.4x the
# bf16 matmul rate.  V projection / S / PV stay bf16 for accuracy.  Inputs are
# pre-scaled on host (e4m3 min-normal is 2^-6, the raw 0.02-sigma weights
# would land in subnormals); the product scale is divided back out during
# PSUM eviction.
# Measured on HW: fp8 DoubleRow QK was both slower (170us vs 151us) and out of
# tolerance (rel err 2.3e-2) — stay on bf16.
FP8_QK = False
FP8E4 = mybir.dt.float8e4
X8S = 16.0    # fmap fp8 pre-scale
W8S = 512.0   # w_qk fp8 pre-scale
OUT_BF16 = True
# f32r broadcast matmul rejected by the BIR verifier ("not rounded to FP32r");
# the reciprocal is cast to bf16 for the K=1 broadcast matmul instead.
F32R_BPS = False

B = 2          # batches per core
HEADS = 4
D = 128
S = 1024       # 32*32 spatial
C = 512        # input channels
CT = C // 128  # contraction tiles
JT = S // 128  # j tiles
NH = 2         # 512-wide halves of the free dim

_CACHED_NC = None
LAST_RESULT = None


def build():
    nc = bacc.Bacc()
    fmap_ext = nc.declare_dram_parameter("fmap", [B, C, S], BF16, isOutput=False)
    w_ext = nc.declare_dram_parameter("w_lhsT", [C, 1536], BF16, isOutput=False)
    embT_ext = nc.declare_dram_parameter("embT", [D, S], F32, isOutput=False)
    out_ext = nc.declare_dram_parameter(
        "out", [B, HEADS * D, S], BF16 if OUT_BF16 else F32, isOutput=True
    )
    if FP8_QK:
        fmap8_ext = nc.declare_dram_parameter("fmap8", [B, C, S], FP8E4, isOutput=False)
        w8_ext = nc.declare_dram_parameter("w8", [C, 1024], FP8E4, isOutput=False)

    heads = [(b, h) for b in range(B) for h in range(HEADS)]
    NHEADS = len(heads)

    with tile.TileContext(nc) as tc:
        with (
            tc.tile_pool(name="const", bufs=1) as const,
            tc.tile_pool(name="ep", bufs=16) as ep,
            tc.tile_pool(name="sump", bufs=12) as sump,
            tc.tile_pool(name="dp", bufs=2) as dp,
            tc.tile_pool(name="orp", bufs=2) as orp,
            tc.tile_pool(name="op", bufs=2) as op,
            tc.tile_pool(name="mm", bufs=2, space="PSUM") as mm,
            tc.tile_pool(name="otp", bufs=1, space="PSUM") as otp,
            tc.tile_pool(name="aux", bufs=2, space="PSUM") as aux,
        ):
            # ---- constants / inputs ----
            w_sb = const.tile([128, CT, 1536], BF16)
            embT_sb = const.tile([D, S], F32)
            x_sb = [const.tile([128, CT, S], BF16, name=f"x{b}") for b in range(B)]
            q_sb = [const.tile([128, HEADS, S], BF16, name=f"q{b}") for b in range(B)]
            k_sb = [const.tile([128, HEADS, S], BF16, name=f"k{b}") for b in range(B)]
            v_sb = [const.tile([128, JT, 512], BF16, name=f"v{b}") for b in range(B)]
            ones_bf = const.tile([128, 1], BF16)
            ones_col = const.tile([1, 128], F32 if F32R_BPS else BF16)
            nc.gpsimd.memset(ones_bf[:], 1.0)
            nc.gpsimd.memset(ones_col[:], 1.0)
            # prime the ACT exp table before the first real activation
            exp_warm = const.tile([1, 1], F32)
            nc.scalar.activation(out=exp_warm[:], in_=ones_bf[:1, :], func=AF.Exp)
            # junk matmuls on memset data: keep the PE busy while the input DMA
            # streams so the HAM clock gate flips to 8/8 before real work
            scratch_w = const.tile([128, 128], BF16)
            nc.vector.memset(scratch_w[:], 1.0)
            for _ in range(WARMUP_MM):
                jt = aux.tile([128, 512], F32, tag="aux", name="junk")
                nc.tensor.matmul(jt[:, 0:128], scratch_w[:], scratch_w[:],
                                 start=True, stop=True)

            # DMA order: K-projection weight columns + batch-0 fmap first so
            # the first S matmuls (head 0) can start ~5us in.
            src_w = w_ext.rearrange("(t p) o -> p t o", p=128)
            src_x = [fmap_ext[b].rearrange("(t p) s -> p t s", p=128) for b in range(B)]
            if FP8_QK:
                x8_sb = [const.tile([128, CT, S], FP8E4, name=f"x8_{b}") for b in range(B)]
                w8_sb = const.tile([128, CT, 1024], FP8E4)
                src_w8 = w8_ext.rearrange("(t p) o -> p t o", p=128)
                src_x8 = [
                    fmap8_ext[b].rearrange("(t p) s -> p t s", p=128) for b in range(B)
                ]
                for kt in range(CT):
                    nc.sync.dma_start(out=w8_sb[:, kt, 512:1024], in_=src_w8[:, kt, 512:1024])
                    nc.sync.dma_start(out=x8_sb[0][:, kt, :], in_=src_x8[0][:, kt, :])
                    if kt == 0:
                        nc.sync.dma_start(out=embT_sb, in_=embT_ext[:])
                for kt in range(CT):
                    nc.sync.dma_start(out=w8_sb[:, kt, 0:512], in_=src_w8[:, kt, 0:512])
                for kt in range(CT):
                    nc.sync.dma_start(out=x_sb[0][:, kt, :], in_=src_x[0][:, kt, :])
                for kt in range(CT):
                    nc.sync.dma_start(out=w_sb[:, kt, 1024:1536], in_=src_w[:, kt, 1024:1536])
                for kt in range(CT):
                    nc.sync.dma_start(out=x8_sb[1][:, kt, :], in_=src_x8[1][:, kt, :])
                for kt in range(CT):
                    nc.sync.dma_start(out=x_sb[1][:, kt, :], in_=src_x[1][:, kt, :])
            else:
                for kt in range(CT):
                    nc.sync.dma_start(out=w_sb[:, kt, 512:1024], in_=src_w[:, kt, 512:1024])
                    nc.sync.dma_start(out=x_sb[0][:, kt, :], in_=src_x[0][:, kt, :])
                    if kt == 0:
                        nc.sync.dma_start(out=embT_sb, in_=embT_ext[:])
                for kt in range(CT):
                    nc.sync.dma_start(out=w_sb[:, kt, 0:512], in_=src_w[:, kt, 0:512])
                for kt in range(CT):
                    nc.sync.dma_start(out=w_sb[:, kt, 1024:1536], in_=src_w[:, kt, 1024:1536])
                for kt in range(CT):
                    nc.sync.dma_start(out=x_sb[1][:, kt, :], in_=src_x[1][:, kt, :])

            # ---- QKV GEMM chunks ([128,512] of the [1536,1024] output) ----
            def emit_qk_chunk(b, m, n):
                t = aux.tile([128, 512], F32, tag="aux", name="qkc")
                if FP8_QK:
                    for p in range(CT // 2):
                        nc.tensor.matmul(
                            t[:, :],
                            w8_sb[:, 2 * p:2 * p + 2, m * 128:(m + 1) * 128],
                            x8_sb[b][:, 2 * p:2 * p + 2, n * 512:(n + 1) * 512],
                            start=(p == 0),
                            stop=(p == CT // 2 - 1),
                            perf_mode=mybir.MatmulPerfMode.DoubleRow,
                        )
                else:
                    for kt in range(CT):
                        nc.tensor.matmul(
                            t[:, :],
                            w_sb[:, kt, m * 128:(m + 1) * 128],
                            x_sb[b][:, kt, n * 512:(n + 1) * 512],
                            start=(kt == 0),
                            stop=(kt == CT - 1),
                        )
                unscale = 1.0 / (X8S * W8S) if FP8_QK else 1.0
                if m < 4:
                    # q eviction on ACT: DVE is the tighter engine globally.
                    # softmax scale folds in here for the fp8 path (it lives
                    # in the bf16 weights otherwise).
                    nc.scalar.activation(
                        out=q_sb[b][:, m, n * 512:(n + 1) * 512], in_=t[:, :],
                        func=AF.Copy,
                        scale=(D ** -0.5) * unscale if FP8_QK else 1.0,
                    )
                elif FP8_QK:
                    nc.vector.scalar_tensor_tensor(
                        out=k_sb[b][:, m - 4, n * 512:(n + 1) * 512],
                        in0=t[:, :],
                        scalar=unscale,
                        in1=embT_sb[:, n * 512:(n + 1) * 512],
                        op0=MUL,
                        op1=mybir.AluOpType.add,
                    )
                else:
                    nc.vector.tensor_add(
                        k_sb[b][:, m - 4, n * 512:(n + 1) * 512],
                        t[:, :],
                        embT_sb[:, n * 512:(n + 1) * 512],
                    )

            def emit_v_chunk(b, j):
                t = aux.tile([128, 512], F32, tag="aux", name="vc")
                for kt in range(CT):
                    nc.tensor.matmul(
                        t[:, :],
                        x_sb[b][:, kt, j * 128:(j + 1) * 128],
                        w_sb[:, kt, 1024:1536],
                        start=(kt == 0),
                        stop=(kt == CT - 1),
                    )
                nc.vector.tensor_copy(v_sb[b][:, j, :], t[:, :])

            chunk_q = []

            def emit_chunks(k):
                for _ in range(min(k, len(chunk_q))):
                    kind, *a = chunk_q.pop(0)
                    (emit_qk_chunk if kind == "qk" else emit_v_chunk)(*a)

            # ---- attention pieces ----
            exps = {}
            tree = {}
            expsum = {}
            recip = {}
            o_raw = {}
            ot = {}

            # MEASURED: fusing exp into [128,2048] pair-activations via one
            # shared 4-bank S tile serialized PE<->ACT (162us vs 145us) —
            # pair pipelining needs 8 banks we don't have.  Per-tile exp.
            def emit_S(hi, j):
                b, h = heads[hi]
                s_ps = mm.tile([128, S], F32, tag="s", name="s_ps")
                for n in range(NH):
                    nc.tensor.matmul(
                        s_ps[:, n * 512:(n + 1) * 512],
                        k_sb[b][:, h, j * 128:(j + 1) * 128],
                        q_sb[b][:, h, n * 512:(n + 1) * 512],
                        start=True,
                        stop=True,
                    )
                return s_ps

            def emit_E(hi, j, s_ps):
                e = ep.tile([128, S], BF16, tag="e", name="exps")
                nc.scalar.activation(out=e[:], in_=s_ps[:], func=AF.Exp)
                exps[(hi, j)] = e

            def tadd(eng, a, b_, nm):
                t = sump.tile([128, S], BF16, tag="tree", name=nm)
                eng.tensor_add(t[:], a[:], b_[:])
                return t

            def emit_tree(hi, j):
                # adder tree over the 8 exp tiles.  GPSIMD ops are ~2.2us each
                # so it only gets the two early leaves; the post-boundary
                # critical chain is two short DVE adds so expsum lands ~1.8us
                # after the final exp pair.
                e = exps
                if j == 1:
                    tree[(hi, 0)] = tadd(nc.gpsimd, e[(hi, 0)], e[(hi, 1)], "g01")
                elif j == 3:
                    tree[(hi, 1)] = tadd(nc.gpsimd, e[(hi, 2)], e[(hi, 3)], "g23")
                elif j == 5:
                    tree[(hi, 2)] = tadd(nc.vector, e[(hi, 4)], e[(hi, 5)], "d45")
                    tree[(hi, 3)] = tadd(nc.gpsimd, tree[(hi, 0)], tree[(hi, 1)], "w0123")
                elif j == 6:
                    tree[(hi, 4)] = tadd(nc.vector, tree[(hi, 2)], e[(hi, 6)], "v456")
                elif j == 7:
                    v7 = tadd(nc.vector, tree[(hi, 4)], e[(hi, 7)], "v4567")
                    expsum[hi] = tadd(nc.vector, tree[(hi, 3)], v7, "es")

            def emit_PV(hi, j):
                b, h = heads[hi]
                if j == 0:
                    ot[hi] = otp.tile([128, S], F32, tag="ot", name="ot_ps")
                for n in range(NH):
                    nc.tensor.matmul(
                        ot[hi][:, n * 512:(n + 1) * 512],
                        v_sb[b][:, j, h * 128:(h + 1) * 128],
                        exps[(hi, j)][:, n * 512:(n + 1) * 512],
                        start=(j == 0),
                        stop=(j == JT - 1),
                    )

            def emit_T1(hi):
                # softmax denominator: column sums via M=1 ones-matmul, then
                # fast reciprocal; runs one window after the exps are done
                es = expsum[hi]
                dh = []
                for n in range(NH):
                    t = aux.tile([128, 512], F32, tag="aux", name="dps")
                    nc.tensor.matmul(
                        t[:1, :], ones_bf[:], es[:, n * 512:(n + 1) * 512],
                        start=True, stop=True,
                    )
                    dh.append(t)
                rf = dp.tile([1, S], F32, tag="rf", name="recip_f")
                for n in range(NH):
                    nc.vector.reciprocal_approx_fast(
                        rf[:, n * 512:(n + 1) * 512], dh[n][:1, :]
                    )
                if F32R_BPS:
                    recip[hi] = rf
                else:
                    # NOTE: this cast on GPSIMD regressed 17us — a 1-partition
                    # tensor leaves 7 of 8 Q7 cores idle.  Keep it on DVE.
                    rb = dp.tile([1, S], BF16, tag="rb", name="recip_b")
                    nc.vector.tensor_copy(rb[:], rf[:])
                    recip[hi] = rb

            def emit_T2(hi, parts=2):
                # evacuate O^T so the single otp slot frees for the next head;
                # split ACT/DVE so it completes in ~0.7us (4 parts for the
                # final head to tighten the drain tail).  For late heads ACT
                # is the binding engine (exp deficit) so DVE takes it all.
                t = orp.tile([128, S], F32, tag="oraw", name="o_raw")
                w = S // parts
                for p in range(parts):
                    lo, hi_ = p * w, (p + 1) * w
                    if p % 2 == 0:
                        nc.scalar.activation(out=t[:, lo:hi_], in_=ot[hi][:, lo:hi_],
                                             func=AF.Copy)
                    else:
                        nc.vector.tensor_copy(t[:, lo:hi_], ot[hi][:, lo:hi_])
                o_raw[hi] = t

            def emit_T3(hi, parts=2):
                # broadcast 1/denom to 128 partitions via K=1 outer product,
                # multiply straight out of PSUM, DMA out per part
                b, h = heads[hi]
                osb = op.tile([128, S], BF16 if OUT_BF16 else F32, tag="o", name="o_sb")
                w = S // parts
                for p in range(parts):
                    lo, hi_ = p * w, (p + 1) * w
                    t = aux.tile([128, 512], F32, tag="aux", name="bps")
                    nc.tensor.matmul(
                        t[:, 0:w], ones_col[:], recip[hi][:, lo:hi_],
                        start=True, stop=True,
                    )
                    nc.vector.tensor_tensor(
                        out=osb[:, lo:hi_],
                        in0=o_raw[hi][:, lo:hi_],
                        in1=t[:, 0:w],
                        op=MUL,
                    )
                    nc.sync.dma_start(
                        out=out_ext[b, h * 128:(h + 1) * 128, lo:hi_],
                        in_=osb[:, lo:hi_],
                    )

            # ---- emission ----
            # prologue: QK(b0) for head 0 first, then head 0's S/exp stream
            # interleaved with the rest of batch 0's QK chunks
            for m in (4, 0):
                for n in range(NH):
                    emit_qk_chunk(0, m, n)
            chunk_q = [("qk", 0, m, n) for m in (5, 1, 6, 2, 7, 3) for n in range(NH)]
            for j in range(JT):
                s_ps = emit_S(0, j)
                emit_E(0, j, s_ps)
                emit_tree(0, j)
                emit_chunks(1 if j % 2 == 0 else 2)
            emit_chunks(len(chunk_q))
            # prime the first two V chunks; the rest stream JIT in window 0
            emit_v_chunk(0, 0)
            emit_v_chunk(0, 1)

            # steady-state head windows.  V-projection chunks stream
            # just-in-time inside windows 0 and 4 (their data isn't needed
            # earlier); QK(b1) fills windows 1-3.  This front-loads nothing
            # into the late bare windows but maximizes the PE work available
            # while ACT's exp stream banks run-ahead.
            win_chunks = {
                1: [("qk", 1, m, n) for m in (4, 0, 5) for n in range(NH)],
                2: [("qk", 1, m, n) for m in (1, 6, 2) for n in range(NH)],
                3: [("qk", 1, m, n) for m in (7, 3) for n in range(NH)],
            }
            jit_v = {0: 0, 4: 1}  # window -> batch whose V chunks stream JIT
            for hi in range(NHEADS):
                chunk_q = win_chunks.get(hi, [])
                if hi > 0:
                    emit_T2(hi - 1)
                emit_chunks(2)
                if hi in jit_v and hi > 0:
                    emit_v_chunk(jit_v[hi], 0)
                    emit_v_chunk(jit_v[hi], 1)
                for j in range(JT):
                    if hi in jit_v and j < JT - 2:
                        emit_v_chunk(jit_v[hi], j + 2)
                    emit_PV(hi, j)
                    if hi + 1 < NHEADS:
                        s_ps = emit_S(hi + 1, j)
                        emit_E(hi + 1, j, s_ps)
                        emit_tree(hi + 1, j)
                    if j == 0 and hi > 0:
                        emit_T3(hi - 1)
                    if j == 2:
                        emit_T1(hi)
                    if j in (2, 4, 5, 6):
                        emit_chunks(1)
                emit_chunks(len(chunk_q))
            emit_T2(NHEADS - 1, parts=4)
            emit_T3(NHEADS - 1, parts=4)
    nc.finalize()
    return nc


def _get_nc():
    global _CACHED_NC
    if _CACHED_NC is None:
        _CACHED_NC = build()
    return _CACHED_NC


def kernel(fmap, w_qkv, height, width):
    fmap = np.ascontiguousarray(np.asarray(fmap, dtype=np.float32))
    w_qkv = np.asarray(w_qkv, dtype=np.float32)
    height = np.asarray(height, dtype=np.float32)
    width = np.asarray(width, dtype=np.float32)

    nb, c, hh, ww = fmap.shape  # (16, 512, 32, 32)
    s = hh * ww
    scale = D ** -0.5

    w_lhsT = np.ascontiguousarray(w_qkv.T).astype(np.float32)  # [512, 1536]
    w_lhsT[:, :512] *= scale  # fold softmax scale into Q projection
    w_lhsT = w_lhsT.astype(ml_dtypes.bfloat16)
    embT = np.ascontiguousarray(
        (height[:, None, :] + width[None, :, :]).reshape(s, D).T
    ).astype(np.float32)  # [128, 1024]

    fm = fmap.reshape(nb, c, s).astype(ml_dtypes.bfloat16)
    nc = _get_nc()
    in_maps = [
        {"fmap": fm[B * i:B * (i + 1)], "w_lhsT": w_lhsT, "embT": embT}
        for i in range(8)
    ]
    if FP8_QK:
        e4m3 = mybir.dt.np(FP8E4)
        fm8 = (fmap.reshape(nb, c, s) * X8S).astype(e4m3)
        w8 = np.ascontiguousarray(w_qkv.T[:, :1024] * W8S).astype(e4m3)
        for i in range(8):
            in_maps[i]["fmap8"] = fm8[B * i:B * (i + 1)]
            in_maps[i]["w8"] = w8

    from concourse.bass_utils import run_bass_kernel_spmd
    res = run_bass_kernel_spmd(nc, in_maps, core_ids=list(range(8)))
    global LAST_RESULT
    LAST_RESULT = res
    out = np.concatenate(
        [np.asarray(r["out"], dtype=np.float32) for r in res.results], axis=0
    )  # (16, 512, 1024)
    return np.ascontiguousarray(out.reshape(nb, HEADS * D, hh, ww)).astype(np.float32)


if __name__ == "__main__":
    rng = np.random.default_rng(0)
    inputs = {
        "fmap": rng.standard_normal((16, 512, 32, 32)).astype(np.float32),
        "w_qkv": (rng.standard_normal((1536, 512)) * 0.02).astype(np.float32),
        "height": (rng.standard_normal((32, 128)) * (128 ** -0.5)).astype(np.float32),
        "width": (rng.standard_normal((32, 128)) * (128 ** -0.5)).astype(np.float32),
    }
    out = kernel(**inputs)
    print(out.shape, out.dtype)


# revision 49
# speedup vs baseline: 1.0188x; 1.0188x over previous
"""Trainium2 Bass kernel for nn_Attention (dense transformer attention over 32x32 fmap).

Math (per batch):
    qkv = w_qkv @ fmap_flat            # [1536, 1024] = [1536,512] @ [512,1024]
    q, k, v per head: [128, 1024] in (d, s) layout
    emb[s, d] = height[x] + width[y];  s = 32*x + y
    sim = (q^T (k + emb^T)) * scale    # scale folded into q weights on host
    out[h*128+d, s] = softmax_j(sim)^T V  computed as O^T = V_jd^T @ expS_T / denom

Sharding: data-parallel over batch, 2 batches per core on 8 cores. No collectives.

Schedule: the ACT engine's exp stream (64 x [128,1024] tiles, ~73us) is the
second-largest engine load after the PE (~102us), so the kernel is built as a
single software pipeline in which the QKV GEMM is chopped into [128,512]-output
chunks and interleaved into the per-head attention windows: during head h's PV
stream the PE also computes head h+1's S tiles (feeding ACT) and a budget of
QKV chunks for the next batch. ACT runs nothing but exp; PSUM evictions go to
DVE, the softmax-denominator adder tree is split across DVE and GPSIMD, and
the 1/denom broadcast is multiplied straight out of PSUM.
"""
import numpy as np
import ml_dtypes

import concourse.bass as bass
import concourse.mybir as mybir
from concourse import bacc
import concourse.tile as tile

F32 = mybir.dt.float32
F32R = mybir.dt.float32r
BF16 = mybir.dt.bfloat16
AF = mybir.ActivationFunctionType
MUL = mybir.AluOpType.mult

WARMUP_MM = 40  # junk matmuls to flip the HAM clock gate to 8/8 during input DMA

# fp8 (e4m3) DoubleRow path for the Q/K projection GEMM: packs the C=512
# contraction as 2 fp8 values per PE cell (virtual 128x256 array), ~1.4x the
# bf16 matmul rate.  V projection / S / PV stay bf16 for accuracy.  Inputs are
# pre-scaled on host (e4m3 min-normal is 2^-6, the raw 0.02-sigma weights
# would land in subnormals); the product scale is divided back out during
# PSUM eviction.
# Measured on HW: fp8 DoubleRow QK was both slower (170us vs 151us) and out of
# tolerance (rel err 2.3e-2) — stay on bf16.
FP8_QK = False
FP8E4 = mybir.dt.float8e4
X8S = 16.0    # fmap fp8 pre-scale
W8S = 512.0   # w_qk fp8 pre-scale
OUT_BF16 = True
# f32r broadcast matmul rejected by the BIR verifier ("not rounded to FP32r");
# the reciprocal is cast to bf16 for the K=1 broadcast matmul instead.
F32R_BPS = False

B = 2          # batches per core
HEADS = 4
D = 128
S = 1024       # 32*32 spatial
C = 512        # input channels
CT = C // 128  # contraction tiles
JT = S // 128  # j tiles
NH = 2         # 512-wide halves of the free dim

_CACHED_NC = None
LAST_RESULT = None


def build():
    nc = bacc.Bacc()
    fmap_ext = nc.declare_dram_parameter("fmap", [B, C, S], BF16, isOutput=False)
    w_ext = nc.declare_dram_parameter("w_lhsT", [C, 1536], BF16, isOutput=False)
    embT_ext = nc.declare_dram_parameter("embT", [D, S], F32, isOutput=False)
    out_ext = nc.declare_dram_parameter(
        "out", [B, HEADS * D, S], BF16 if OUT_BF16 else F32, isOutput=True
    )
    if FP8_QK:
        fmap8_ext = nc.declare_dram_parameter("fmap8", [B, C, S], FP8E4, isOutput=False)
        w8_ext = nc.declare_dram_parameter("w8", [C, 1024], FP8E4, isOutput=False)

    heads = [(b, h) for b in range(B) for h in range(HEADS)]
    NHEADS = len(heads)

    with tile.TileContext(nc) as tc:
        with (
            tc.tile_pool(name="const", bufs=1) as const,
            tc.tile_pool(name="ep", bufs=16) as ep,
            tc.tile_pool(name="sump", bufs=12) as sump,
            tc.tile_pool(name="dp", bufs=2) as dp,
            tc.tile_pool(name="orp", bufs=2) as orp,
            tc.tile_pool(name="op", bufs=2) as op,
            tc.tile_pool(name="mm", bufs=2, space="PSUM") as mm,
            tc.tile_pool(name="otp", bufs=1, space="PSUM") as otp,
            tc.tile_pool(name="aux", bufs=2, space="PSUM") as aux,
        ):
            # ---- constants / inputs ----
            w_sb = const.tile([128, CT, 1536], BF16)
            embT_sb = const.tile([D, S], F32)
            x_sb = [const.tile([128, CT, S], BF16, name=f"x{b}") for b in range(B)]
            q_sb = [const.tile([128, HEADS, S], BF16, name=f"q{b}") for b in range(B)]
            k_sb = [const.tile([128, HEADS, S], BF16, name=f"k{b}") for b in range(B)]
            v_sb = [const.tile([128, JT, 512], BF16, name=f"v{b}") for b in range(B)]
            ones_bf = const.tile([128, 1], BF16)
            ones_col = const.tile([1, 128], F32 if F32R_BPS else BF16)
            nc.gpsimd.memset(ones_bf[:], 1.0)
            nc.gpsimd.memset(ones_col[:], 1.0)
            # prime the ACT exp table before the first real activation
            exp_warm = const.tile([1, 1], F32)
            nc.scalar.activation(out=exp_warm[:], in_=ones_bf[:1, :], func=AF.Exp)
            # junk matmuls on memset data: keep the PE busy while the input DMA
            # streams so the HAM clock gate flips to 8/8 before real work
            scratch_w = const.tile([128, 128], BF16)
            nc.vector.memset(scratch_w[:], 1.0)
            for _ in range(WARMUP_MM):
                jt = aux.tile([128, 512], F32, tag="aux", name="junk")
                nc.tensor.matmul(jt[:, 0:128], scratch_w[:], scratch_w[:],
                                 start=True, stop=True)

            # DMA order: K-projection weight columns + batch-0 fmap first so
            # the first S matmuls (head 0) can start ~5us in.
            src_w = w_ext.rearrange("(t p) o -> p t o", p=128)
            src_x = [fmap_ext[b].rearrange("(t p) s -> p t s", p=128) for b in range(B)]
            if FP8_QK:
                x8_sb = [const.tile([128, CT, S], FP8E4, name=f"x8_{b}") for b in range(B)]
                w8_sb = const.tile([128, CT, 1024], FP8E4)
                src_w8 = w8_ext.rearrange("(t p) o -> p t o", p=128)
                src_x8 = [
                    fmap8_ext[b].rearrange("(t p) s -> p t s", p=128) for b in range(B)
                ]
                for kt in range(CT):
                    nc.sync.dma_start(out=w8_sb[:, kt, 512:1024], in_=src_w8[:, kt, 512:1024])
                    nc.sync.dma_start(out=x8_sb[0][:, kt, :], in_=src_x8[0][:, kt, :])
                    if kt == 0:
                        nc.sync.dma_start(out=embT_sb, in_=embT_ext[:])
                for kt in range(CT):
                    nc.sync.dma_start(out=w8_sb[:, kt, 0:512], in_=src_w8[:, kt, 0:512])
                for kt in range(CT):
                    nc.sync.dma_start(out=x_sb[0][:, kt, :], in_=src_x[0][:, kt, :])
                for kt in range(CT):
                    nc.sync.dma_start(out=w_sb[:, kt, 1024:1536], in_=src_w[:, kt, 1024:1536])
                for kt in range(CT):
                    nc.sync.dma_start(out=x8_sb[1][:, kt, :], in_=src_x8[1][:, kt, :])
                for kt in range(CT):
                    nc.sync.dma_start(out=x_sb[1][:, kt, :], in_=src_x[1][:, kt, :])
            else:
                for kt in range(CT):
                    nc.sync.dma_start(out=w_sb[:, kt, 512:1024], in_=src_w[:, kt, 512:1024])
                    nc.sync.dma_start(out=x_sb[0][:, kt, :], in_=src_x[0][:, kt, :])
                    if kt == 0:
                        nc.sync.dma_start(out=embT_sb, in_=embT_ext[:])
                for kt in range(CT):
                    nc.sync.dma_start(out=w_sb[:, kt, 0:512], in_=src_w[:, kt, 0:512])
                for kt in range(CT):
                    nc.sync.dma_start(out=w_sb[:, kt, 1024:1536], in_=src_w[:, kt, 1024:1536])
                for kt in range(CT):
                    nc.sync.dma_start(out=x_sb[1][:, kt, :], in_=src_x[1][:, kt, :])

            # ---- QKV GEMM chunks ([128,512] of the [1536,1024] output) ----
            def emit_qk_chunk(b, m, n):
                t = aux.tile([128, 512], F32, tag="aux", name="qkc")
                if FP8_QK:
                    for p in range(CT // 2):
                        nc.tensor.matmul(
                            t[:, :],
                            w8_sb[:, 2 * p:2 * p + 2, m * 128:(m + 1) * 128],
                            x8_sb[b][:, 2 * p:2 * p + 2, n * 512:(n + 1) * 512],
                            start=(p == 0),
                            stop=(p == CT // 2 - 1),
                            perf_mode=mybir.MatmulPerfMode.DoubleRow,
                        )
                else:
                    for kt in range(CT):
                        nc.tensor.matmul(
                            t[:, :],
                            w_sb[:, kt, m * 128:(m + 1) * 128],
                            x_sb[b][:, kt, n * 512:(n + 1) * 512],
                            start=(kt == 0),
                            stop=(kt == CT - 1),
                        )
                unscale = 1.0 / (X8S * W8S) if FP8_QK else 1.0
                if m < 4:
                    # q eviction on ACT: DVE is the tighter engine globally.
                    # softmax scale folds in here for the fp8 path (it lives
                    # in the bf16 weights otherwise).
                    nc.scalar.activation(
                        out=q_sb[b][:, m, n * 512:(n + 1) * 512], in_=t[:, :],
                        func=AF.Copy,
                        scale=(D ** -0.5) * unscale if FP8_QK else 1.0,
                    )
                elif FP8_QK:
                    nc.vector.scalar_tensor_tensor(
                        out=k_sb[b][:, m - 4, n * 512:(n + 1) * 512],
                        in0=t[:, :],
                        scalar=unscale,
                        in1=embT_sb[:, n * 512:(n + 1) * 512],
                        op0=MUL,
                        op1=mybir.AluOpType.add,
                    )
                else:
                    nc.vector.tensor_add(
                        k_sb[b][:, m - 4, n * 512:(n + 1) * 512],
                        t[:, :],
                        embT_sb[:, n * 512:(n + 1) * 512],
                    )

            def emit_v_chunk(b, j):
                t = aux.tile([128, 512], F32, tag="aux", name="vc")
                for kt in range(CT):
                    nc.tensor.matmul(
                        t[:, :],
                        x_sb[b][:, kt, j * 128:(j + 1) * 128],
                        w_sb[:, kt, 1024:1536],
                        start=(kt == 0),
                        stop=(kt == CT - 1),
                    )
                nc.vector.tensor_copy(v_sb[b][:, j, :], t[:, :])

            chunk_q = []

            def emit_chunks(k):
                for _ in range(min(k, len(chunk_q))):
                    kind, *a = chunk_q.pop(0)
                    (emit_qk_chunk if kind == "qk" else emit_v_chunk)(*a)

            # ---- attention pieces ----
            exps = {}
            tree = {}
            expsum = {}
            recip = {}
            o_raw = {}
            ot = {}

            # MEASURED: fusing exp into [128,2048] pair-activations via one
            # shared 4-bank S tile serialized PE<->ACT (162us vs 145us) —
            # pair pipelining needs 8 banks we don't have.  Per-tile exp.
            def emit_S(hi, j):
                b, h = heads[hi]
                s_ps = mm.tile([128, S], F32, tag="s", name="s_ps")
                for n in range(NH):
                    nc.tensor.matmul(
                        s_ps[:, n * 512:(n + 1) * 512],
                        k_sb[b][:, h, j * 128:(j + 1) * 128],
                        q_sb[b][:, h, n * 512:(n + 1) * 512],
                        start=True,
                        stop=True,
                    )
                return s_ps

            def emit_E(hi, j, s_ps):
                e = ep.tile([128, S], BF16, tag="e", name="exps")
                nc.scalar.activation(out=e[:], in_=s_ps[:], func=AF.Exp)
                exps[(hi, j)] = e

            def tadd(eng, a, b_, nm):
                t = sump.tile([128, S], BF16, tag="tree", name=nm)
                eng.tensor_add(t[:], a[:], b_[:])
                return t

            def emit_tree(hi, j):
                # adder tree over the 8 exp tiles.  GPSIMD ops are ~2.2us each
                # so it only gets the two early leaves; the post-boundary
                # critical chain is two short DVE adds so expsum lands ~1.8us
                # after the final exp pair.
                e = exps
                if j == 1:
                    tree[(hi, 0)] = tadd(nc.gpsimd, e[(hi, 0)], e[(hi, 1)], "g01")
                elif j == 3:
                    tree[(hi, 1)] = tadd(nc.gpsimd, e[(hi, 2)], e[(hi, 3)], "g23")
                elif j == 5:
                    tree[(hi, 2)] = tadd(nc.vector, e[(hi, 4)], e[(hi, 5)], "d45")
                    tree[(hi, 3)] = tadd(nc.gpsimd, tree[(hi, 0)], tree[(hi, 1)], "w0123")
                elif j == 6:
                    tree[(hi, 4)] = tadd(nc.vector, tree[(hi, 2)], e[(hi, 6)], "v456")
                elif j == 7:
                    v7 = tadd(nc.vector, tree[(hi, 4)], e[(hi, 7)], "v4567")
                    expsum[hi] = tadd(nc.vector, tree[(hi, 3)], v7, "es")

            def emit_PV(hi, j):
                b, h = heads[hi]
                if j == 0:
                    ot[hi] = otp.tile([128, S], F32, tag="ot", name="ot_ps")
                for n in range(NH):
                    nc.tensor.matmul(
                        ot[hi][:, n * 512:(n + 1) * 512],
                        v_sb[b][:, j, h * 128:(h + 1) * 128],
                        exps[(hi, j)][:, n * 512:(n + 1) * 512],
                        start=(j == 0),
                        stop=(j == JT - 1),
                    )

            def emit_T1(hi):
                # softmax denominator: column sums via M=1 ones-matmul, then
                # fast reciprocal; runs one window after the exps are done
                es = expsum[hi]
                dh = []
                for n in range(NH):
                    t = aux.tile([128, 512], F32, tag="aux", name="dps")
                    nc.tensor.matmul(
                        t[:1, :], ones_bf[:], es[:, n * 512:(n + 1) * 512],
                        start=True, stop=True,
                    )
                    dh.append(t)
                rf = dp.tile([1, S], F32, tag="rf", name="recip_f")
                for n in range(NH):
                    nc.vector.reciprocal_approx_fast(
                        rf[:, n * 512:(n + 1) * 512], dh[n][:1, :]
                    )
                if F32R_BPS:
                    recip[hi] = rf
                else:
                    # NOTE: this cast on GPSIMD regressed 17us — a 1-partition
                    # tensor leaves 7 of 8 Q7 cores idle.  Keep it on DVE.
                    rb = dp.tile([1, S], BF16, tag="rb", name="recip_b")
                    nc.vector.tensor_copy(rb[:], rf[:])
                    recip[hi] = rb

            def emit_T2(hi, parts=2):
                # evacuate O^T so the single otp slot frees for the next head;
                # split ACT/DVE so it completes in ~0.7us (4 parts for the
                # final head to tighten the drain tail).  For late heads ACT
                # is the binding engine (exp deficit) so DVE takes it all.
                t = orp.tile([128, S], F32, tag="oraw", name="o_raw")
                w = S // parts
                for p in range(parts):
                    lo, hi_ = p * w, (p + 1) * w
                    if p % 2 == 0:
                        nc.scalar.activation(out=t[:, lo:hi_], in_=ot[hi][:, lo:hi_],
                                             func=AF.Copy)
                    else:
                        nc.vector.tensor_copy(t[:, lo:hi_], ot[hi][:, lo:hi_])
                o_raw[hi] = t

            def emit_T3(hi, parts=2):
                # broadcast 1/denom to 128 partitions via K=1 outer product,
                # multiply straight out of PSUM, DMA out per part
                b, h = heads[hi]
                osb = op.tile([128, S], BF16 if OUT_BF16 else F32, tag="o", name="o_sb")
                w = S // parts
                for p in range(parts):
                    lo, hi_ = p * w, (p + 1) * w
                    t = aux.tile([128, 512], F32, tag="aux", name="bps")
                    nc.tensor.matmul(
                        t[:, 0:w], ones_col[:], recip[hi][:, lo:hi_],
                        start=True, stop=True,
                    )
                    nc.vector.tensor_tensor(
                        out=osb[:, lo:hi_],
                        in0=o_raw[hi][:, lo:hi_],
                        in1=t[:, 0:w],
                        op=MUL,
                    )
                    nc.sync.dma_start(
                        out=out_ext[b, h * 128:(h + 1) * 128, lo:hi_],
                        in_=osb[:, lo:hi_],
                    )

            # ---- emission ----
            # prologue: QK(b0) for head 0 first, then head 0's S/exp stream
            # interleaved with the rest of batch 0's QK chunks
            for m in (4, 0):
                for n in range(NH):
                    emit_qk_chunk(0, m, n)
            chunk_q = [("qk", 0, m, n) for m in (5, 1, 6, 2, 7, 3) for n in range(NH)]
            chunk_q += [("v", 0, j) for j in range(JT)]
            for j in range(JT):
                s_ps = emit_S(0, j)
                emit_E(0, j, s_ps)
                emit_tree(0, j)
                emit_chunks(2 if j % 2 == 0 else 3)
            emit_chunks(len(chunk_q))

            # steady-state head windows
            chunk_q = [("qk", 1, m, n) for m in (4, 0, 5, 1, 6, 2, 7, 3) for n in range(NH)]
            chunk_q += [("v", 1, j) for j in range(JT)]
            for hi in range(NHEADS):
                if hi > 0:
                    emit_T2(hi - 1)
                emit_chunks(2)
                for j in range(JT):
                    emit_PV(hi, j)
                    if hi + 1 < NHEADS:
                        s_ps = emit_S(hi + 1, j)
                        emit_E(hi + 1, j, s_ps)
                        emit_tree(hi + 1, j)
                    if j == 0 and hi > 0:
                        emit_T3(hi - 1)
                    if j == 2:
                        emit_T1(hi)
                    if j in (2, 4, 5, 6):
                        emit_chunks(1)
            emit_T2(NHEADS - 1, parts=4)
            emit_T3(NHEADS - 1, parts=4)
    nc.finalize()
    return nc


def _get_nc():
    global _CACHED_NC
    if _CACHED_NC is None:
        _CACHED_NC = build()
    return _CACHED_NC


def kernel(fmap, w_qkv, height, width):
    fmap = np.ascontiguousarray(np.asarray(fmap, dtype=np.float32))
    w_qkv = np.asarray(w_qkv, dtype=np.float32)
    height = np.asarray(height, dtype=np.float32)
    width = np.asarray(width, dtype=np.float32)

    nb, c, hh, ww = fmap.shape  # (16, 512, 32, 32)
    s = hh * ww
    scale = D ** -0.5

    w_lhsT = np.ascontiguousarray(w_qkv.T).astype(np.float32)  # [512, 1536]
    w_lhsT[:, :512] *= scale  # fold softmax scale into Q projection
    w_lhsT = w_lhsT.astype(ml_dtypes.bfloat16)
    embT = np.ascontiguousarray(
        (height[:, None, :] + width[None, :, :]).reshape(s, D).T
    ).astype(np.float32)  # [128, 1024]

    fm = fmap.reshape(nb, c, s).astype(ml_dtypes.bfloat16)
    nc = _get_nc()
    in_maps = [
        {"fmap": fm[B * i:B * (i + 1)], "w_lhsT": w_lhsT, "embT": embT}
        for i in range(8)
    ]
    if FP8_QK:
        e4m3 = mybir.dt.np(FP8E4)
        fm8 = (fmap.reshape(nb, c, s) * X8S).astype(e4m3)
        w8 = np.ascontiguousarray(w_qkv.T[:, :1024] * W8S).astype(e4m3)
        for i in range(8):
            in_maps[i]["fmap8"] = fm8[B * i:B * (i + 1)]
            in_maps[i]["w8"] = w8

    from concourse.bass_utils import run_bass_kernel_spmd
    res = run_bass_kernel_spmd(nc, in_maps, core_ids=list(range(8)))
    global LAST_RESULT
    LAST_RESULT = res
    out = np.concatenate(
        [np.asarray(r["out"], dtype=np.float32) for r in res.results], axis=0
    )  # (16, 512, 1024)
    return np.ascontiguousarray(out.reshape(nb, HEADS * D, hh, ww)).astype(np.float32)


if __name__ == "__main__":
    rng = np.random.default_rng(0)
    inputs = {
        "fmap": rng.standard_normal((16, 512, 32, 32)).astype(np.float32),
        "w_qkv": (rng.standard_normal((1536, 512)) * 0.02).astype(np.float32),
        "height": (rng.standard_normal((32, 128)) * (128 ** -0.5)).astype(np.float32),
        "width": (rng.standard_normal((32, 128)) * (128 ** -0.5)).astype(np.float32),
    }
    out = kernel(**inputs)
    print(out.shape, out.dtype)
